# revision 41
# baseline (speedup 1.0000x reference)
"""Multi-head attention (B=2, S=4096, D=512, H=8) on 8 TRN2 NeuronCores.

Sharding: data-parallel over (batch, query-chunk). Core i handles batch
i//4 and query rows (i%4)*1024 .. +1024 of that batch. Each core
computes Q projection for its query chunk, K/V projections for the full
batch (redundantly, 4 cores per batch), full attention for all 8 heads
over its queries, and the output projection for its rows. Output slices
are disjoint -> no collectives; host just concatenates.

Per-core device pipeline (transposed "d-major" layout, bf16 matmuls):
  1. Transposing DMAs (bf16 xbar mode) load x^T on the sync queue while
     weights/biases stream in parallel on the scalar-engine HWDGE queue.
  2. Q^T/K^T = W^T.T @ x^T; V = x^T.T @ Wv^T (natural layout), stored
     bf16 with a ones-column per head (V_aug).
  3. Per (head-pair, q-tile 512, k-chunk 128): scores^T [k,q] via 2
     row-packed matmuls, one ACT exp [128,1024] psum->sbuf (scale=1/8),
     2 attn@V matmuls lhsT=[V_h|1] [128,65] -> psum [65,512]; row 64
     accumulates the softmax denominator. scores/exp for k+1 are
     emitted before attn@V of k (software pipeline).
  4. All deferred work (projections for later head-pairs, softmax
     normalization, output projection) is queued as ~1-matmul closures
     and drained one budget unit per k-iteration so the PE load stays
     smooth and the ACT engine (the exp is the per-core floor:
     256 x 1.34us) never starves.
  5. Normalize per (pair, q-tile): fast approx reciprocal of the
     denominator row, rank-1 broadcast matmul, scalar_tensor_tensor.
  6. Output projection runs per head-pair into a PSUM tile and is
     accumulated into a persistent SBUF buffer by the vector engine, so
     the final tail after the last exp is only a few microseconds.

Engines in steady state: ACT saturated by the exp; PE runs scores,
attn@V and the dripped projection matmuls just under the ACT rate.
"""

import numpy as np
import ml_dtypes

import concourse.bass as bass
import concourse.tile as tile
from concourse import bacc, mybir
from concourse.bass_utils import run_bass_kernel_spmd

F32 = mybir.dt.float32
F32R = mybir.dt.float32r
BF16 = mybir.dt.bfloat16
MUL = mybir.AluOpType.mult
ADD = mybir.AluOpType.add

B, S, D, H = 2, 4096, 512, 8
HD = D // H  # 64
NCORES = 8
QCH = B * S // NCORES  # 1024 query rows per core
TKV = S  # 4096 kv rows per core
IC = D // 128  # 4 contraction chunks
OC = D // 128  # 4 output chunks
QT = 512  # q tile (psum bank limit in fp32)
NQT = QCH // QT  # 2
KCH = TKV // 128  # 32 k chunks


def _build_program():
    nc = bacc.Bacc(
        "TRN2",
        target_bir_lowering=False,
        debug=False,
        enable_asserts=False,
        num_devices=NCORES,
    )
    xq = nc.dram_tensor("xq", [QCH, D], BF16, kind="ExternalInput").ap()
    xkv = nc.dram_tensor("xkv", [TKV, D], BF16, kind="ExternalInput").ap()
    wqt = nc.dram_tensor("wqt", [D, D], BF16, kind="ExternalInput").ap()
    wkt = nc.dram_tensor("wkt", [D, D], BF16, kind="ExternalInput").ap()
    wvt = nc.dram_tensor("wvt", [D, D], BF16, kind="ExternalInput").ap()
    wos = nc.dram_tensor("wos", [HD, H, D], BF16, kind="ExternalInput").ap()
    bqs = nc.dram_tensor("bqs", [128, OC], F32, kind="ExternalInput").ap()
    bks = nc.dram_tensor("bks", [128, OC], F32, kind="ExternalInput").ap()
    bvb = nc.dram_tensor("bvb", [128, D], F32, kind="ExternalInput").ap()
    bob = nc.dram_tensor("bob", [128, D], F32, kind="ExternalInput").ap()
    out = nc.dram_tensor("out", [QCH, D], F32, kind="ExternalOutput").ap()
    # DRAM bounce buffer for transposing softmax-denominator rows
    dsc = nc.dram_tensor("denscratch", [2, 2, QT], F32).ap()

    with tile.TileContext(nc) as tc:
        with (
            tc.tile_pool(name="consts", bufs=1) as consts,
            tc.tile_pool(name="persist", bufs=1) as persist,
            tc.tile_pool(name="pt", bufs=4) as pt_pool,
            tc.tile_pool(name="aot", bufs=2) as aot_pool,
            # PSUM (8 banks): "sc" scores 2x2, "acc" 2x1 (proj, pb,
            # fin), "po" 2x1 attn-out accumulators.
            tc.tile_pool(name="ps_sc", bufs=2, space="PSUM") as sc_pool,
            tc.tile_pool(name="ps_acc", bufs=2, space="PSUM") as acc_pool,
            tc.tile_pool(name="ps_po", bufs=2, space="PSUM") as po_pool,
        ):
            # ---- constants ----
            # Denominator path: po row HD (the ones-column sum) is DMA'd
            # PSUM->DRAM->SBUF so it lands partition-major [128, t4]; the
            # reciprocal then runs 128-wide and normalization folds into
            # the per-head output-projection accumulate as a per-partition
            # scalar multiply. 2 rotating slots x 2 heads.
            dent = consts.tile([128, 2, 2, QT // 128], F32, name="dent")
            dentr = consts.tile([128, 2, 2, QT // 128], F32, name="dentr")
            den_sb = consts.tile([1, 2, QT], F32, name="den_sb")
            den_slot = [0]
            # scratch for the tail block's ACT-assisted normalize
            fintmp = consts.tile([128, 2, D], F32, name="fintmp")

            # ---- persistent activations ----
            # x_kv^T split: segment 0 small (gates the prefix), rest in one
            # tile so its transpose lands in 4 big DMAs instead of 12
            xtk0 = persist.tile([128, IC, 1024], BF16, name="xtk0")
            xtkr = persist.tile([128, IC, TKV - 1024], BF16, name="xtkr")

            def xtk_ap(t0, i, w):
                # x^T block at t-columns [t0, t0+w), contraction chunk i
                if t0 < 1024:
                    return xtk0[:, i, t0 : t0 + w]
                return xtkr[:, i, t0 - 1024 : t0 - 1024 + w]
            xtq = persist.tile([128, IC, QCH], BF16)  # x_q^T
            kt = persist.tile([128, OC, TKV], BF16)  # K^T [o-in-chunk, c, t]
            qt = persist.tile([128, OC, QCH], BF16)  # Q^T
            # V_aug: [t-in-chunk, t-chunk, head, 64 V cols + ones col]
            v_sb = persist.tile([128, KCH, H, HD + 1], BF16)
            nc.vector.memset(v_sb[:, :, :, HD : HD + 1], 1.0)
            # output accumulator [q-in-chunk, qi, t4, D] f32
            out_acc = persist.tile([128, NQT, QT // 128, D], F32)

            # ---- DMA (single sync HWDGE queue; each DMA serializes at
            # ~1.4us, so order + count dominate startup): xq transposes,
            # Q weights, x_kv segment 0, K/V weights, the remaining 3072
            # t-rows as 4 big transposes, then output weights.
            for c in range(IC):
                nc.sync.dma_start_transpose(
                    xtq[:, c, :], xq[:, c * 128 : (c + 1) * 128]
                )
            wq_sb = consts.tile([128, IC, D], BF16)
            nc.sync.dma_start(wq_sb, wqt.rearrange("(c p) o -> p c o", p=128))
            bq_sb = consts.tile([128, OC], F32)
            nc.sync.dma_start(bq_sb, bqs)
            SEG = 1024
            for c in range(IC):
                nc.sync.dma_start_transpose(
                    xtk0[:, c, :], xkv[0:SEG, c * 128 : (c + 1) * 128]
                )
            wk_sb = consts.tile([128, IC, D], BF16)
            nc.sync.dma_start(wk_sb, wkt.rearrange("(c p) o -> p c o", p=128))
            bk_sb = consts.tile([128, OC], F32)
            nc.sync.dma_start(bk_sb, bks)
            wv_sb = consts.tile([128, IC, D], BF16)
            nc.sync.dma_start(wv_sb, wvt.rearrange("(c p) o -> p c o", p=128))
            bvb_sb = consts.tile([128, D], F32)
            nc.sync.dma_start(bvb_sb, bvb)
            bob_sb = consts.tile([128, D], F32)
            nc.sync.dma_start(bob_sb, bob)
            for c in range(IC):
                nc.sync.dma_start_transpose(
                    xtkr[:, c, :], xkv[SEG:TKV, c * 128 : (c + 1) * 128]
                )
            wo_sb = consts.tile([HD, H, D], BF16)
            nc.sync.dma_start(wo_sb, wos)

            # ---- projection units ----
            def k_unit(c, tt):
                # atomic 512-col K^T unit (used in the first block, where
                # closure interleaving with v_unit allocs must stay
                # one-tile-at-a-time)
                ps = acc_pool.tile([128, 512], F32, tag="acc", name=f"k{c}_{tt}")
                for i in range(IC):
                    nc.tensor.matmul(
                        ps,
                        wk_sb[:, i, c * 128 : (c + 1) * 128],
                        xtk_ap(tt * 512, i, 512),
                        start=(i == 0),
                        stop=(i == IC - 1),
                    )
                nc.vector.tensor_scalar_add(
                    kt[:, c, tt * 512 : (tt + 1) * 512], ps, bk_sb[:, c : c + 1]
                )

            def v_unit(j):
                # V rows for t-chunk j, all heads: [128 t, 512 d] + bias
                ps = acc_pool.tile([128, D], F32, tag="acc", name=f"v{j}")
                for i in range(IC):
                    nc.tensor.matmul(
                        ps,
                        xtk_ap(j * 128, i, 128),
                        wv_sb[:, i, :],
                        start=(i == 0),
                        stop=(i == IC - 1),
                    )
                nc.vector.tensor_add(
                    v_sb[:, j, :, 0:HD],
                    ps.rearrange("p (h d) -> p h d", h=H),
                    bvb_sb.rearrange("p (h d) -> p h d", h=H),
                )

            def q_pair_closures(c):
                # q chunks for both 512-col tiles, weights shared per i
                st = {}

                def mk(i):
                    def go():
                        if i == 0:
                            st["a"] = acc_pool.tile(
                                [128, 512], F32, tag="acc", name=f"qa{c}"
                            )
                            st["b"] = acc_pool.tile(
                                [128, 512], F32, tag="acc", name=f"qb{c}"
                            )
                        w = wq_sb[:, i, c * 128 : (c + 1) * 128]
                        nc.tensor.matmul(
                            st["a"], w, xtq[:, i, 0:512],
                            start=(i == 0), stop=(i == IC - 1),
                        )
                        nc.tensor.matmul(
                            st["b"], w, xtq[:, i, 512:1024],
                            start=(i == 0), stop=(i == IC - 1),
                        )
                        if i == IC - 1:
                            nc.vector.tensor_scalar_add(
                                qt[:, c, 0:512], st["a"], bq_sb[:, c : c + 1]
                            )
                            nc.vector.tensor_scalar_add(
                                qt[:, c, 512:1024], st["b"], bq_sb[:, c : c + 1]
                            )
                    return (2, go)

                return [mk(i) for i in range(IC)]

            def k_pair_closures(c, p):
                # K^T chunk c, t-cols [p*1024, (p+1)*1024), weights shared
                st = {}

                def mk(i):
                    def go():
                        if i == 0:
                            st["a"] = acc_pool.tile(
                                [128, 512], F32, tag="acc", name=f"ka{c}_{p}"
                            )
                            st["b"] = acc_pool.tile(
                                [128, 512], F32, tag="acc", name=f"kb{c}_{p}"
                            )
                        w = wk_sb[:, i, c * 128 : (c + 1) * 128]
                        nc.tensor.matmul(
                            st["a"], w, xtk_ap(p * 1024, i, 512),
                            start=(i == 0), stop=(i == IC - 1),
                        )
                        nc.tensor.matmul(
                            st["b"], w, xtk_ap(p * 1024 + 512, i, 512),
                            start=(i == 0), stop=(i == IC - 1),
                        )
                        if i == IC - 1:
                            t0 = p * 1024
                            nc.vector.tensor_scalar_add(
                                kt[:, c, t0 : t0 + 512], st["a"],
                                bk_sb[:, c : c + 1],
                            )
                            nc.vector.tensor_scalar_add(
                                kt[:, c, t0 + 512 : t0 + 1024], st["b"],
                                bk_sb[:, c : c + 1],
                            )
                    return (2, go)

                return [mk(i) for i in range(IC)]

            # ---- normalize + output projection closures for one block ----
            def normfin_closures(c, qi):
                st = {}
                sl = den_slot[0]
                den_slot[0] = (sl + 1) % 2

                def norm_a(hh, po):
                    # inline at block end: copy the (unnormalized) head
                    # output to bf16 and bounce the denominator row
                    # through DRAM into partition-major layout. Frees the
                    # po psum banks within ~2us, no slow ops in the path.
                    if hh == 0:
                        st["aot"] = aot_pool.tile(
                            [HD, 2, QT], BF16, name=f"aot{c}_{qi}"
                        )
                    # ship the denominator row first so its DRAM round
                    # trip overlaps the aot copy
                    nc.vector.tensor_copy(den_sb[0:1, hh, :], po[HD : HD + 1, :])
                    nc.sync.dma_start(dsc[sl, hh, :], den_sb[0:1, hh, :])
                    nc.sync.dma_start(
                        dent[:, sl, hh, :],
                        dsc[sl, hh, :].rearrange("(t p) -> p t", p=128),
                    )
                    nc.vector.tensor_copy(st["aot"][:, hh, :], po[0:HD, :])

                def mk_recip():
                    def go():
                        with nc.allow_low_precision(reason="denom recip"):
                            nc.vector.reciprocal(
                                dentr[:, sl, :, :], dent[:, sl, :, :]
                            )
                    return (0, go)

                def mk_fin(t4, hh):
                    def go():
                        ps = acc_pool.tile(
                            [128, D], F32, tag="acc", name=f"f{c}_{qi}_{t4}_{hh}"
                        )
                        nc.tensor.matmul(
                            ps,
                            st["aot"][:, hh, t4 * 128 : (t4 + 1) * 128],
                            wo_sb[:, 2 * c + hh, :],
                            start=True,
                            stop=True,
                        )
                        dst = out_acc[:, qi, t4, :]
                        den = dentr[:, sl, hh, t4 : t4 + 1]
                        last = c == H // 2 - 1
                        if last and qi == NQT - 1:
                            # tail block: ACT is idle after the final exp,
                            # so split normalize-multiply (ACT) from the
                            # accumulate (DVE) to halve the serial tail
                            tsl = (2 * t4 + hh) % 2
                            tmp = fintmp[:, tsl, :]
                            nc.scalar.activation(
                                tmp, ps,
                                mybir.ActivationFunctionType.Copy,
                                scale=den,
                            )
                            nc.vector.tensor_add(dst, dst, tmp)
                        else:
                            other = bob_sb if (c == 0 and hh == 0) else dst
                            nc.vector.scalar_tensor_tensor(
                                dst, ps, den, other, op0=MUL, op1=ADD
                            )
                        if last and hh == 1:
                            t0 = qi * QT + t4 * 128
                            nc.sync.dma_start(out[t0 : t0 + 128, :], dst)
                    return (1, go)

                return norm_a, mk_recip, mk_fin

            # ---- pending-work queue: (mm_cost, closure), drained with a
            # per-iteration budget so PE load stays smooth.
            pending = []
            budget = [0.0]

            def drain(rate):
                budget[0] = min(budget[0] + rate, 4.0)
                while pending and pending[0][0] <= budget[0]:
                    cost, fn = pending.pop(0)
                    fn()
                    budget[0] -= cost

            # ---- prefix: minimum projections before attention (only
            # x^T segment 0 required, so PE starts as soon as the first
            # transposes land)
            for cl in q_pair_closures(0):
                cl[1]()
            for cl in k_pair_closures(0, 0):
                cl[1]()
            for j in range(4):
                v_unit(j)
            # remaining K chunk-0 units drain inside the first block
            pending.extend(
                (4, (lambda tt=tt: k_unit(0, tt))) for tt in range(2, 8)
            )

            # ---- attention ----
            for c in range(H // 2):
                for qi in range(NQT):
                    if qi == 1 and c + 1 < H // 2:
                        pending.extend(q_pair_closures(c + 1))
                        for p in range(4):
                            pending.extend(k_pair_closures(c + 1, p))
                    qs = qi * QT
                    po = [
                        po_pool.tile(
                            [HD + 1, QT], F32, tag="po", name=f"po{c}_{qi}_{hh}"
                        )
                        for hh in range(2)
                    ]

                    def scores_exp(k, qs=qs, c=c):
                        pss = sc_pool.tile([128, 2, QT], F32, tag="sc")
                        for hh in range(2):
                            off = hh * HD
                            nc.tensor.matmul(
                                pss[:, hh, :],
                                kt[off : off + HD, c, k * 128 : (k + 1) * 128],
                                qt[off : off + HD, c, qs : qs + QT],
                                start=True,
                                stop=True,
                            )
                        ptile = pt_pool.tile([128, 2, QT], BF16, tag="pt")
                        nc.scalar.activation(
                            ptile, pss, mybir.ActivationFunctionType.Exp,
                            scale=1.0 / np.sqrt(HD),
                        )
                        return ptile

                    first = c == 0 and qi == 0
                    ptile = scores_exp(0)
                    for k in range(KCH):
                        nxt = scores_exp(k + 1) if k + 1 < KCH else None
                        for hh in range(2):
                            nc.tensor.matmul(
                                po[hh],
                                v_sb[:, k, 2 * c + hh, :],
                                ptile[:, hh, :],
                                start=(k == 0),
                                stop=(k == KCH - 1),
                            )
                        ptile = nxt
                        if first and k < 28:
                            v_unit(k + 4)
                        drain(1.0 if k < 28 else 2.0)
                    # queue normalization + output projection for this block
                    norm_a, mk_recip, mk_f = normfin_closures(c, qi)
                    norm_a(0, po[0])
                    norm_a(1, po[1])
                    pending.append(mk_recip())
                    for t4 in range(QT // 128):
                        for hh in range(2):
                            pending.append(mk_f(t4, hh))
            while pending:
                pending.pop(0)[1]()

    nc.compile()
    return nc


_NC_CACHE = None


def _get_program():
    global _NC_CACHE
    if _NC_CACHE is None:
        _NC_CACHE = _build_program()
    return _NC_CACHE


def prepare_in_maps(x, Wq, bq, Wk, bk, Wv, bv, Wo, bo):
    bf = ml_dtypes.bfloat16
    x = np.ascontiguousarray(np.asarray(x, dtype=np.float32)).astype(bf)
    sh = {
        "wqt": np.ascontiguousarray(np.asarray(Wq, np.float32).T).astype(bf),
        "wkt": np.ascontiguousarray(np.asarray(Wk, np.float32).T).astype(bf),
        "wvt": np.ascontiguousarray(np.asarray(Wv, np.float32).T).astype(bf),
        "wos": np.ascontiguousarray(
            np.asarray(Wo, np.float32).T.reshape(H, HD, D).transpose(1, 0, 2)
        ).astype(bf),
        "bqs": np.ascontiguousarray(np.asarray(bq, np.float32).reshape(OC, 128).T),
        "bks": np.ascontiguousarray(np.asarray(bk, np.float32).reshape(OC, 128).T),
        "bvb": np.ascontiguousarray(
            np.broadcast_to(np.asarray(bv, np.float32), (128, D))
        ),
        "bob": np.ascontiguousarray(
            np.broadcast_to(np.asarray(bo, np.float32), (128, D))
        ),
    }
    in_maps = []
    for core in range(NCORES):
        b = core // (NCORES // B)
        qs = (core % (NCORES // B)) * QCH
        m = dict(sh)
        m["xq"] = np.ascontiguousarray(x[b, qs : qs + QCH, :])
        m["xkv"] = np.ascontiguousarray(x[b])
        in_maps.append(m)
    return in_maps


def assemble(results):
    out = np.empty((B, S, D), dtype=np.float32)
    for core in range(NCORES):
        b = core // (NCORES // B)
        qs = (core % (NCORES // B)) * QCH
        out[b, qs : qs + QCH, :] = results[core]["out"]
    return out


def kernel(x, Wq, bq, Wk, bk, Wv, bv, Wo, bo):
    in_maps = prepare_in_maps(x, Wq, bq, Wk, bk, Wv, bv, Wo, bo)
    nc = _get_program()
    res = run_bass_kernel_spmd(nc, in_maps, core_ids=list(range(NCORES)))
    return assemble(res.results)


# revision 43
# speedup vs baseline: 1.1625x; 1.1625x over previous
"""Multi-head attention (B=2, S=4096, D=512, H=8) on 8 TRN2 NeuronCores.

Sharding: data-parallel over (batch, query-chunk). Core i handles batch
i//4 and query rows (i%4)*1024 .. +1024 of that batch. Each core
computes Q projection for its query chunk, K/V projections for the full
batch (redundantly, 4 cores per batch), full attention for all 8 heads
over its queries, and the output projection for its rows. Output slices
are disjoint -> no collectives; host just concatenates.

Per-core device pipeline (transposed "d-major" layout, bf16 matmuls):
  1. Transposing DMAs (bf16 xbar mode) load x^T on the sync queue while
     weights/biases stream in parallel on the scalar-engine HWDGE queue.
  2. Q^T/K^T = W^T.T @ x^T; V = x^T.T @ Wv^T (natural layout), stored
     bf16 with a ones-column per head (V_aug).
  3. Per (head-pair, q-tile 512, k-chunk 128): scores^T [k,q] via 2
     row-packed matmuls, one ACT exp [128,1024] psum->sbuf (scale=1/8),
     2 attn@V matmuls lhsT=[V_h|1] [128,65] -> psum [65,512]; row 64
     accumulates the softmax denominator. scores/exp for k+1 are
     emitted before attn@V of k (software pipeline).
  4. All deferred work (projections for later head-pairs, softmax
     normalization, output projection) is queued as ~1-matmul closures
     and drained one budget unit per k-iteration so the PE load stays
     smooth and the ACT engine (the exp is the per-core floor:
     256 x 1.34us) never starves.
  5. Normalize per (pair, q-tile): fast approx reciprocal of the
     denominator row, rank-1 broadcast matmul, scalar_tensor_tensor.
  6. Output projection runs per head-pair into a PSUM tile and is
     accumulated into a persistent SBUF buffer by the vector engine, so
     the final tail after the last exp is only a few microseconds.

Engines in steady state: ACT saturated by the exp; PE runs scores,
attn@V and the dripped projection matmuls just under the ACT rate.
"""

import numpy as np
import ml_dtypes

import concourse.bass as bass
import concourse.tile as tile
from concourse import bacc, mybir
from concourse.bass_utils import run_bass_kernel_spmd

F32 = mybir.dt.float32
F32R = mybir.dt.float32r
BF16 = mybir.dt.bfloat16
MUL = mybir.AluOpType.mult
ADD = mybir.AluOpType.add

B, S, D, H = 2, 4096, 512, 8
HD = D // H  # 64
NCORES = 8
QCH = B * S // NCORES  # 1024 query rows per core
TKV = S  # 4096 kv rows per core
IC = D // 128  # 4 contraction chunks
OC = D // 128  # 4 output chunks
QT = 512  # q tile (psum bank limit in fp32)
NQT = QCH // QT  # 2
KCH = TKV // 128  # 32 k chunks


def _build_program():
    nc = bacc.Bacc(
        "TRN2",
        target_bir_lowering=False,
        debug=False,
        enable_asserts=False,
        num_devices=NCORES,
    )
    xq = nc.dram_tensor("xq", [QCH, D], BF16, kind="ExternalInput").ap()
    xkv = nc.dram_tensor("xkv", [TKV, D], BF16, kind="ExternalInput").ap()
    wqt = nc.dram_tensor("wqt", [D, D], BF16, kind="ExternalInput").ap()
    wkt = nc.dram_tensor("wkt", [D, D], BF16, kind="ExternalInput").ap()
    wvt = nc.dram_tensor("wvt", [D, D], BF16, kind="ExternalInput").ap()
    wos = nc.dram_tensor("wos", [HD, H, D], BF16, kind="ExternalInput").ap()
    bqs = nc.dram_tensor("bqs", [128, OC], F32, kind="ExternalInput").ap()
    bks = nc.dram_tensor("bks", [128, OC], F32, kind="ExternalInput").ap()
    bvb = nc.dram_tensor("bvb", [128, D], F32, kind="ExternalInput").ap()
    bob = nc.dram_tensor("bob", [128, D], F32, kind="ExternalInput").ap()
    out = nc.dram_tensor("out", [QCH, D], F32, kind="ExternalOutput").ap()
    # DRAM bounce buffer for transposing softmax-denominator rows
    dsc = nc.dram_tensor("denscratch", [2, 2, QT], F32).ap()

    with tile.TileContext(nc) as tc:
        with (
            tc.tile_pool(name="consts", bufs=1) as consts,
            tc.tile_pool(name="persist", bufs=1) as persist,
            tc.tile_pool(name="pt", bufs=4) as pt_pool,
            tc.tile_pool(name="aot", bufs=2) as aot_pool,
            # declared last: keeps earlier pools at layout-sensitive
            # addresses (the exp's SBUF write speed depends on placement)
            tc.tile_pool(name="tailtmp", bufs=1) as tail_pool,
            # PSUM (8 banks): "sc" scores 2x2, "acc" 2x1 (proj, pb,
            # fin), "po" 2x1 attn-out accumulators.
            tc.tile_pool(name="ps_sc", bufs=2, space="PSUM") as sc_pool,
            tc.tile_pool(name="ps_acc", bufs=2, space="PSUM") as acc_pool,
            tc.tile_pool(name="ps_po", bufs=2, space="PSUM") as po_pool,
        ):
            # ---- constants ----
            # Denominator path: po row HD (the ones-column sum) is DMA'd
            # PSUM->DRAM->SBUF so it lands partition-major [128, t4]; the
            # reciprocal then runs 128-wide and normalization folds into
            # the per-head output-projection accumulate as a per-partition
            # scalar multiply. 2 rotating slots x 2 heads.
            dent = consts.tile([128, 2, 2, QT // 128], F32, name="dent")
            dentr = consts.tile([128, 2, 2, QT // 128], F32, name="dentr")
            den_sb = consts.tile([1, 2, QT], F32, name="den_sb")
            den_slot = [0]
            # scratch for the tail block's ACT-assisted normalize
            fintmp = tail_pool.tile([128, 2, D], F32, name="fintmp")

            # ---- persistent activations ----
            # x_kv^T split: segment 0 small (gates the prefix), rest in one
            # tile so its transpose lands in 4 big DMAs instead of 12
            xtk0 = persist.tile([128, IC, 1024], BF16, name="xtk0")
            xtkr = persist.tile([128, IC, TKV - 1024], BF16, name="xtkr")

            def xtk_ap(t0, i, w):
                # x^T block at t-columns [t0, t0+w), contraction chunk i
                if t0 < 1024:
                    return xtk0[:, i, t0 : t0 + w]
                return xtkr[:, i, t0 - 1024 : t0 - 1024 + w]
            xtq = persist.tile([128, IC, QCH], BF16)  # x_q^T
            kt = persist.tile([128, OC, TKV], BF16)  # K^T [o-in-chunk, c, t]
            qt = persist.tile([128, OC, QCH], BF16)  # Q^T
            # V_aug: [t-in-chunk, t-chunk, head, 64 V cols + ones col]
            v_sb = persist.tile([128, KCH, H, HD + 1], BF16)
            nc.vector.memset(v_sb[:, :, :, HD : HD + 1], 1.0)
            # output accumulator [q-in-chunk, qi, t4, D] f32
            out_acc = persist.tile([128, NQT, QT // 128, D], F32)

            # ---- DMA (single sync HWDGE queue; each DMA serializes at
            # ~1.4us, so order + count dominate startup): xq transposes,
            # Q weights, x_kv segment 0, K/V weights, the remaining 3072
            # t-rows as 4 big transposes, then output weights.
            for c in range(IC):
                nc.sync.dma_start_transpose(
                    xtq[:, c, :], xq[:, c * 128 : (c + 1) * 128]
                )
            wq_sb = consts.tile([128, IC, D], BF16)
            nc.sync.dma_start(wq_sb, wqt.rearrange("(c p) o -> p c o", p=128))
            bq_sb = consts.tile([128, OC], F32)
            nc.sync.dma_start(bq_sb, bqs)
            SEG = 1024
            for c in range(IC):
                nc.sync.dma_start_transpose(
                    xtk0[:, c, :], xkv[0:SEG, c * 128 : (c + 1) * 128]
                )
            wk_sb = consts.tile([128, IC, D], BF16)
            nc.sync.dma_start(wk_sb, wkt.rearrange("(c p) o -> p c o", p=128))
            bk_sb = consts.tile([128, OC], F32)
            nc.sync.dma_start(bk_sb, bks)
            wv_sb = consts.tile([128, IC, D], BF16)
            nc.sync.dma_start(wv_sb, wvt.rearrange("(c p) o -> p c o", p=128))
            bvb_sb = consts.tile([128, D], F32)
            nc.sync.dma_start(bvb_sb, bvb)
            bob_sb = consts.tile([128, D], F32)
            nc.sync.dma_start(bob_sb, bob)
            for c in range(IC):
                nc.sync.dma_start_transpose(
                    xtkr[:, c, :], xkv[SEG:TKV, c * 128 : (c + 1) * 128]
                )
            wo_sb = consts.tile([HD, H, D], BF16)
            nc.sync.dma_start(wo_sb, wos)

            # ---- projection units ----
            def k_unit(c, tt):
                # atomic 512-col K^T unit (used in the first block, where
                # closure interleaving with v_unit allocs must stay
                # one-tile-at-a-time)
                ps = acc_pool.tile([128, 512], F32, tag="acc", name=f"k{c}_{tt}")
                for i in range(IC):
                    nc.tensor.matmul(
                        ps,
                        wk_sb[:, i, c * 128 : (c + 1) * 128],
                        xtk_ap(tt * 512, i, 512),
                        start=(i == 0),
                        stop=(i == IC - 1),
                    )
                nc.vector.tensor_scalar_add(
                    kt[:, c, tt * 512 : (tt + 1) * 512], ps, bk_sb[:, c : c + 1]
                )

            def v_unit(j):
                # V rows for t-chunk j, all heads: [128 t, 512 d] + bias
                ps = acc_pool.tile([128, D], F32, tag="acc", name=f"v{j}")
                for i in range(IC):
                    nc.tensor.matmul(
                        ps,
                        xtk_ap(j * 128, i, 128),
                        wv_sb[:, i, :],
                        start=(i == 0),
                        stop=(i == IC - 1),
                    )
                nc.vector.tensor_add(
                    v_sb[:, j, :, 0:HD],
                    ps.rearrange("p (h d) -> p h d", h=H),
                    bvb_sb.rearrange("p (h d) -> p h d", h=H),
                )

            def q_pair_closures(c):
                # q chunks for both 512-col tiles, weights shared per i
                st = {}

                def mk(i):
                    def go():
                        if i == 0:
                            st["a"] = acc_pool.tile(
                                [128, 512], F32, tag="acc", name=f"qa{c}"
                            )
                            st["b"] = acc_pool.tile(
                                [128, 512], F32, tag="acc", name=f"qb{c}"
                            )
                        w = wq_sb[:, i, c * 128 : (c + 1) * 128]
                        nc.tensor.matmul(
                            st["a"], w, xtq[:, i, 0:512],
                            start=(i == 0), stop=(i == IC - 1),
                        )
                        nc.tensor.matmul(
                            st["b"], w, xtq[:, i, 512:1024],
                            start=(i == 0), stop=(i == IC - 1),
                        )
                        if i == IC - 1:
                            nc.vector.tensor_scalar_add(
                                qt[:, c, 0:512], st["a"], bq_sb[:, c : c + 1]
                            )
                            nc.vector.tensor_scalar_add(
                                qt[:, c, 512:1024], st["b"], bq_sb[:, c : c + 1]
                            )
                    return (2, go)

                return [mk(i) for i in range(IC)]

            def k_pair_closures(c, p):
                # K^T chunk c, t-cols [p*1024, (p+1)*1024), weights shared
                st = {}

                def mk(i):
                    def go():
                        if i == 0:
                            st["a"] = acc_pool.tile(
                                [128, 512], F32, tag="acc", name=f"ka{c}_{p}"
                            )
                            st["b"] = acc_pool.tile(
                                [128, 512], F32, tag="acc", name=f"kb{c}_{p}"
                            )
                        w = wk_sb[:, i, c * 128 : (c + 1) * 128]
                        nc.tensor.matmul(
                            st["a"], w, xtk_ap(p * 1024, i, 512),
                            start=(i == 0), stop=(i == IC - 1),
                        )
                        nc.tensor.matmul(
                            st["b"], w, xtk_ap(p * 1024 + 512, i, 512),
                            start=(i == 0), stop=(i == IC - 1),
                        )
                        if i == IC - 1:
                            t0 = p * 1024
                            nc.vector.tensor_scalar_add(
                                kt[:, c, t0 : t0 + 512], st["a"],
                                bk_sb[:, c : c + 1],
                            )
                            nc.vector.tensor_scalar_add(
                                kt[:, c, t0 + 512 : t0 + 1024], st["b"],
                                bk_sb[:, c : c + 1],
                            )
                    return (2, go)

                return [mk(i) for i in range(IC)]

            # ---- normalize + output projection closures for one block ----
            def normfin_closures(c, qi):
                st = {}
                sl = den_slot[0]
                den_slot[0] = (sl + 1) % 2

                def norm_a(hh, po):
                    # inline at block end: copy the (unnormalized) head
                    # output to bf16 and bounce the denominator row
                    # through DRAM into partition-major layout. Frees the
                    # po psum banks within ~2us, no slow ops in the path.
                    if hh == 0:
                        st["aot"] = aot_pool.tile(
                            [HD, 2, QT], BF16, name=f"aot{c}_{qi}"
                        )
                    # ship the denominator row first so its DRAM round
                    # trip overlaps the aot copy
                    nc.vector.tensor_copy(den_sb[0:1, hh, :], po[HD : HD + 1, :])
                    nc.sync.dma_start(dsc[sl, hh, :], den_sb[0:1, hh, :])
                    nc.sync.dma_start(
                        dent[:, sl, hh, :],
                        dsc[sl, hh, :].rearrange("(t p) -> p t", p=128),
                    )
                    nc.vector.tensor_copy(st["aot"][:, hh, :], po[0:HD, :])

                def mk_recip():
                    def go():
                        with nc.allow_low_precision(reason="denom recip"):
                            nc.vector.reciprocal(
                                dentr[:, sl, :, :], dent[:, sl, :, :]
                            )
                    return (0, go)

                def mk_fin(t4, hh):
                    def go():
                        ps = acc_pool.tile(
                            [128, D], F32, tag="acc", name=f"f{c}_{qi}_{t4}_{hh}"
                        )
                        nc.tensor.matmul(
                            ps,
                            st["aot"][:, hh, t4 * 128 : (t4 + 1) * 128],
                            wo_sb[:, 2 * c + hh, :],
                            start=True,
                            stop=True,
                        )
                        dst = out_acc[:, qi, t4, :]
                        den = dentr[:, sl, hh, t4 : t4 + 1]
                        last = c == H // 2 - 1
                        if last and qi == NQT - 1:
                            # tail block: ACT is idle after the final exp,
                            # so split normalize-multiply (ACT) from the
                            # accumulate (DVE) to halve the serial tail
                            tsl = (2 * t4 + hh) % 2
                            tmp = fintmp[:, tsl, :]
                            nc.scalar.activation(
                                tmp, ps,
                                mybir.ActivationFunctionType.Copy,
                                scale=den,
                            )
                            nc.vector.tensor_add(dst, dst, tmp)
                        else:
                            other = bob_sb if (c == 0 and hh == 0) else dst
                            nc.vector.scalar_tensor_tensor(
                                dst, ps, den, other, op0=MUL, op1=ADD
                            )
                        if last and hh == 1:
                            t0 = qi * QT + t4 * 128
                            nc.sync.dma_start(out[t0 : t0 + 128, :], dst)
                    return (1, go)

                return norm_a, mk_recip, mk_fin

            # ---- pending-work queue: (mm_cost, closure), drained with a
            # per-iteration budget so PE load stays smooth.
            pending = []
            budget = [0.0]

            def drain(rate):
                budget[0] = min(budget[0] + rate, 4.0)
                while pending and pending[0][0] <= budget[0]:
                    cost, fn = pending.pop(0)
                    fn()
                    budget[0] -= cost

            # ---- prefix: minimum projections before attention (only
            # x^T segment 0 required, so PE starts as soon as the first
            # transposes land)
            for cl in q_pair_closures(0):
                cl[1]()
            for cl in k_pair_closures(0, 0):
                cl[1]()
            for j in range(4):
                v_unit(j)
            # remaining K chunk-0 units drain inside the first block
            pending.extend(
                (4, (lambda tt=tt: k_unit(0, tt))) for tt in range(2, 8)
            )

            # ---- attention ----
            for c in range(H // 2):
                for qi in range(NQT):
                    if qi == 1 and c + 1 < H // 2:
                        pending.extend(q_pair_closures(c + 1))
                        for p in range(4):
                            pending.extend(k_pair_closures(c + 1, p))
                    qs = qi * QT
                    po = [
                        po_pool.tile(
                            [HD + 1, QT], F32, tag="po", name=f"po{c}_{qi}_{hh}"
                        )
                        for hh in range(2)
                    ]

                    def scores_exp(k, qs=qs, c=c):
                        pss = sc_pool.tile([128, 2, QT], F32, tag="sc")
                        for hh in range(2):
                            off = hh * HD
                            nc.tensor.matmul(
                                pss[:, hh, :],
                                kt[off : off + HD, c, k * 128 : (k + 1) * 128],
                                qt[off : off + HD, c, qs : qs + QT],
                                start=True,
                                stop=True,
                            )
                        ptile = pt_pool.tile([128, 2, QT], BF16, tag="pt")
                        nc.scalar.activation(
                            ptile, pss, mybir.ActivationFunctionType.Exp,
                            scale=1.0 / np.sqrt(HD),
                        )
                        return ptile

                    first = c == 0 and qi == 0
                    ptile = scores_exp(0)
                    for k in range(KCH):
                        nxt = scores_exp(k + 1) if k + 1 < KCH else None
                        for hh in range(2):
                            nc.tensor.matmul(
                                po[hh],
                                v_sb[:, k, 2 * c + hh, :],
                                ptile[:, hh, :],
                                start=(k == 0),
                                stop=(k == KCH - 1),
                            )
                        ptile = nxt
                        if first and k < 28:
                            v_unit(k + 4)
                        drain(1.0 if k < 28 else 2.0)
                    # queue normalization + output projection for this block
                    norm_a, mk_recip, mk_f = normfin_closures(c, qi)
                    norm_a(0, po[0])
                    norm_a(1, po[1])
                    pending.append(mk_recip())
                    for t4 in range(QT // 128):
                        for hh in range(2):
                            pending.append(mk_f(t4, hh))
            while pending:
                pending.pop(0)[1]()

    nc.compile()
    return nc


_NC_CACHE = None


def _get_program():
    global _NC_CACHE
    if _NC_CACHE is None:
        _NC_CACHE = _build_program()
    return _NC_CACHE


def prepare_in_maps(x, Wq, bq, Wk, bk, Wv, bv, Wo, bo):
    bf = ml_dtypes.bfloat16
    x = np.ascontiguousarray(np.asarray(x, dtype=np.float32)).astype(bf)
    sh = {
        "wqt": np.ascontiguousarray(np.asarray(Wq, np.float32).T).astype(bf),
        "wkt": np.ascontiguousarray(np.asarray(Wk, np.float32).T).astype(bf),
        "wvt": np.ascontiguousarray(np.asarray(Wv, np.float32).T).astype(bf),
        "wos": np.ascontiguousarray(
            np.asarray(Wo, np.float32).T.reshape(H, HD, D).transpose(1, 0, 2)
        ).astype(bf),
        "bqs": np.ascontiguousarray(np.asarray(bq, np.float32).reshape(OC, 128).T),
        "bks": np.ascontiguousarray(np.asarray(bk, np.float32).reshape(OC, 128).T),
        "bvb": np.ascontiguousarray(
            np.broadcast_to(np.asarray(bv, np.float32), (128, D))
        ),
        "bob": np.ascontiguousarray(
            np.broadcast_to(np.asarray(bo, np.float32), (128, D))
        ),
    }
    in_maps = []
    for core in range(NCORES):
        b = core // (NCORES // B)
        qs = (core % (NCORES // B)) * QCH
        m = dict(sh)
        m["xq"] = np.ascontiguousarray(x[b, qs : qs + QCH, :])
        m["xkv"] = np.ascontiguousarray(x[b])
        in_maps.append(m)
    return in_maps


def assemble(results):
    out = np.empty((B, S, D), dtype=np.float32)
    for core in range(NCORES):
        b = core // (NCORES // B)
        qs = (core % (NCORES // B)) * QCH
        out[b, qs : qs + QCH, :] = results[core]["out"]
    return out


def kernel(x, Wq, bq, Wk, bk, Wv, bv, Wo, bo):
    in_maps = prepare_in_maps(x, Wq, bq, Wk, bk, Wv, bv, Wo, bo)
    nc = _get_program()
    res = run_bass_kernel_spmd(nc, in_maps, core_ids=list(range(NCORES)))
    return assemble(res.results)
